# revision 47
# baseline (speedup 1.0000x reference)
"""Causal multi-head attention block (B=2, T=2048, C=1024, H=16) on 8 TRN2
NeuronCores.

Sharding: Megatron-style tensor parallel over heads for QKV+attention
(core r owns heads {2r, 2r+1} = feature rows [128r, 128r+128) of q/k/v),
then a token-sharded output projection via AllToAll.

Differences vs the 225-241us baseline (driven by perfetto analysis):

- The two local heads' S = kT.T @ qT matmuls are PACKED onto the PE
  array concurrently: head A's stationary [64d, 128kt] sits at array
  rows 0-63 (tile_position (0,0), inferred from base partitions), head
  B's at rows 64-127 ((64,0)) -> the two 512-col matmuls overlap almost
  fully (row-group concurrency), halving S cost vs the serial-head
  baseline.  One st PSUM tile [128, 2, 512] holds both heads (2 banks),
  one exp instruction covers both (same 80-exp ACT cadence as before).
- Attention walks k-tiles once per (b, jq) for BOTH heads; PV stays
  per-head serial (the [64|1] ones-row stationary that also accumulates
  the softmax denominator cannot column-pack: 2*(64+1) > 128 PSUM
  partitions).
- THREE AllToAlls: batch 0 whole (fired at ~55% through attention),
  batch 1 in halves (H0 fired at ~75%, H1 at the end).  The platform
  CC is latency-dominated (~20 us per op regardless of 256/512 KB) and
  its stream only becomes usable ~70+ us in, so a tiny DUMMY collective
  fired right after the prologue absorbs the one-time bootstrap +
  rendezvous cost (~10-20 us) off the critical path.  ALL o-projection
  matmuls stay at the tail (mid-attention po fillers gated on a
  collective stall the in-order queues - measured -70 us): po(b0) +
  po(b1,H0) cover A2A(b1,H1); only that A2A's remainder and po(b1,H1)
  (~4 us) are exposed.  Core r owns tokens [256r, +256) of batch 0 and
  [hh*1024 + 128r, +128) of each batch-1 half.
- Startup: only the two ones-columns of v_sb are memset (not the whole
  0.5 MB tile), mneg/ident stay off the x-DMA critical path, and the
  first projection is gated only by w-qkv + x-chunk0 DMAs (~300 GB/s
  burst); no dummy-matmul pacing anywhere (PE is the pacer now - real
  fillers only; ACT exp latency hides under the 1-deep S/PV skew).

Engine/queue roles: x chunks + non-gating evictions on gpsimd (SWDGE);
w / yf loads / out stores on sync; collective-gating evictions on
scalar (HWDGE); proj copies / yn math on vector; A2A triggers gpsimd.
"""

import numpy as np
import ml_dtypes

import concourse.bacc as bacc
import concourse.mybir as mybir
import concourse.tile as tile
from concourse.bass_utils import run_bass_kernel_spmd
from concourse.masks import make_identity

N_CORES = 8
B, T, C, H = 2, 2048, 1024, 16
D = 64                # head dim
HL = H // N_CORES     # heads per core = 2
DL = HL * D           # local feature dim = 128
TT = B * T            # 4096 tokens total
P = 128
NCH = C // P          # 8 contraction chunks
QCH = 512             # projection token-chunk (moving free dim)
NQC = T // QCH        # 4 q-chunks per batch entry
NKT = T // P          # 16 k-tiles per batch entry
NHALF = 2             # collectives per batch
THALF = T // NHALF    # 1024 tokens per half
TPC = THALF // N_CORES  # 128: tokens per core per (batch, half)
SCALE = 1.0 / np.sqrt(D)

BF = mybir.dt.bfloat16
F32 = mybir.dt.float32
AF = mybir.ActivationFunctionType

WQKV = 3 * NCH * DL           # 3072 cols of packed q/k/v shards
WO = NCH * NCH * P            # 8192 cols of packed full Wo
WCOLS = WQKV + WO             # 11264

# act-pacing constant: exp cost ~ (172 + FD)/1.2 ns; PE ~1.95 GHz under
# the fleet 13/16 power throttle -> act period in PE cycles ~ 1.63x.
# Slightly over-provisioned so fillers flow in-stream.
ACT_RATIO = 1.9
ACT_FIXED = 300


def build_graph():
    nc = bacc.Bacc("TRN2", target_bir_lowering=False, debug=False)

    # [p, ci, t] with c = ci*128 + p: one contiguous DMA per token chunk
    xT = nc.dram_tensor("xT", [P, NCH * TT], BF, kind="ExternalInput")
    # q/k/v shards [p, w, ci, m] + full Wo [p, om, ci, m], pre-packed
    wall = nc.dram_tensor("wall", [P, WCOLS], BF, kind="ExternalInput")
    # (m, om, b, t): feature om*128+m, batch b; t<128 -> token
    # [r*128, +128) of half 0, t>=128 -> same of half 1
    out = nc.dram_tensor("out", [P, NCH, B, 2 * TPC], BF,
                         kind="ExternalOutput")

    with tile.TileContext(nc) as tc:
        with (
            tc.tile_pool(name="sb", bufs=1) as sb,
            tc.tile_pool(name="ps", bufs=1, space="PSUM") as ps,
            tc.tile_pool(name="dram", bufs=1, space="DRAM") as dram,
        ):
            # ---- weight + x loads (the qkv third + x chunk 0 gate the
            # first projection; Wo defers past the prologue) ----
            w_sb = sb.tile([P, WCOLS], BF, name="w_sb")
            # w split per projection; wq rides the otherwise-idle
            # scalar HWDGE queue so it doesn't queue behind x chunk 0
            # (first q-proj only waits for its own 256 KB)
            # wq in two halves: the first q-proj fillers only touch
            # ci 0-3, so they start as soon as the first 128 KB lands
            nc.scalar.dma_start(w_sb[:, 0:WQKV // 6], wall[:, 0:WQKV // 6])
            nc.scalar.dma_start(w_sb[:, WQKV // 6:WQKV // 3],
                                wall[:, WQKV // 6:WQKV // 3])
            for pc in range(1, 3):
                csl = slice(pc * (WQKV // 3), (pc + 1) * (WQKV // 3))
                nc.sync.dma_start(w_sb[:, csl], wall[:, csl])
            w3 = w_sb[:, 0:WQKV].rearrange("p (w a m) -> p w a m", w=3, a=NCH)
            wq_sb, wk_sb, wv_sb = (w3[:, i] for i in range(3))
            # tail-po weights live in a vector-copied clone: a compute
            # op's completion semaphore is precise, while a DMA-written
            # region's release threshold rides round-robin-shared lanes
            # and can transitively wait on a collective (measured
            # -17..-32 us stalls on the tail weight loads)
            wo_cp = sb.tile([P, WO], BF, name="wo_cp")
            wo_sb = wo_cp[:].rearrange("p (o a m) -> p o a m", o=NCH, a=NCH)

            xT_sb = sb.tile([P, NCH, TT], BF, name="xT_sb")
            xTv = xT[:].rearrange("p (a t) -> p a t", a=NCH)
            # x chunk 0 split by ci halves for the same reason
            nc.gpsimd.dma_start(xT_sb[:, 0:4, 0:QCH], xTv[:, 0:4, 0:QCH])
            nc.gpsimd.dma_start(xT_sb[:, 4:8, 0:QCH], xTv[:, 4:8, 0:QCH])

            ident = sb.tile([P, P], BF, name="ident")
            make_identity(nc, ident)
            wsrc = sb.tile([P, QCH], BF, name="wsrc")
            nc.vector.memset(wsrc[:], 0.5)

            nc.gpsimd.dma_start(xT_sb[:, :, QCH:2 * QCH],
                                xTv[:, :, QCH:2 * QCH])

            # strictly-lower-triangular -1e9 (k > q) for diagonal blocks
            mneg = sb.tile([P, P], BF, name="mneg")
            nc.gpsimd.memset(mneg[:], 0.0)
            nc.gpsimd.affine_select(
                out=mneg[:], in_=mneg[:],
                compare_op=mybir.AluOpType.is_ge,
                fill=-1e9, base=0, channel_multiplier=-1, pattern=[[1, P]],
            )

            def warm(n):
                for _ in range(n):
                    wdst = ps.tile([P, QCH], F32, tag="fx", bufs=2,
                                   name="wdst")
                    nc.tensor.matmul(wdst[:], ident[:], wsrc[:],
                                     start=True, stop=True)

            qT_sb = sb.tile([P, TT], BF, name="qT_sb")
            kT_sb = sb.tile([P, TT], BF, name="kT_sb")
            vT_sb = sb.tile([P, TT], BF, name="vT_sb")
            # v in natural layout, packed per 128-token tile as
            # [headA(64) | 1 | headB(64) | 1] -> 130 columns; only the
            # two ones-columns are memset (the rest is overwritten by
            # the v transposes)
            v_sb = sb.tile([P, TT // P, 2 * (D + 1)], BF, name="v_sb")
            for h in range(HL):
                nc.gpsimd.memset(v_sb[:, :, h * (D + 1) + D], 1.0)
            for tch in range(2, 8):
                tsl = slice(tch * QCH, (tch + 1) * QCH)
                nc.gpsimd.dma_start(xT_sb[:, :, tsl], xTv[:, :, tsl])

            # AllToAll buffers: per batch, 8 blocks of [128 feats,
            # 256 tok] (core r <- tokens [256r, +256))
            ain = [dram.tile([N_CORES, DL, 2 * TPC], BF, name=f"ain{b}")
                   for b in range(B)]
            aout = [dram.tile([N_CORES, DL, 2 * TPC], BF, name=f"aout{b}")
                    for b in range(B)]

            # ---- projection micro-fillers, per-projection parts so
            # the drip-feed map can hand e.g. tch7's k/v to (1,3) ----
            WDSTS = {"q": (wq_sb, qT_sb), "k": (wk_sb, kT_sb),
                     "v": (wv_sb, vT_sb)}

            def proj_part(tch, which):
                # q copies on vector (prompt, gates the next phase's S);
                # k/v on gpsimd — the vector queue backlogs ~10 us at
                # phase boundaries otherwise (each DVE op pays a DRAIN)
                wsb, dst = WDSTS[which]
                ceng = nc.vector
                tsl = slice(tch * QCH, (tch + 1) * QCH)
                cell = {}
                fillers = []
                def mk(ci0):
                    def f():
                        if ci0 == 0:
                            cell["pj"] = ps.tile([P, QCH], F32, tag="fx",
                                                 bufs=2, name="pj")
                        pj = cell["pj"]
                        for ci in (ci0, ci0 + 1):
                            nc.tensor.matmul(
                                pj[:], wsb[:, ci, :], xT_sb[:, ci, tsl],
                                start=(ci == 0), stop=(ci == NCH - 1),
                            )
                        if ci0 == NCH - 2:
                            ceng.tensor_copy(dst[:, tsl], pj[:])
                    return f
                for ci0 in range(0, NCH, 2):
                    fillers.append((1024, mk(ci0)))
                return fillers

            def vt_part(tch):
                fillers = []
                for t32 in range(tch * 4, tch * 4 + 4):
                    def vt(t32=t32):
                        tr = ps.tile([P, P], BF, tag="fx", bufs=2,
                                     name="tr")
                        nc.tensor.transpose(
                            tr[:], vT_sb[:, t32 * P:(t32 + 1) * P], ident[:]
                        )
                        out_ap = v_sb[:, t32, :].rearrange(
                            "p (h x) -> p h x", h=HL
                        )[:, :, 0:D]
                        in_ap = tr[:].rearrange("p (h x) -> p h x", h=HL)
                        nc.vector.tensor_copy(out_ap, in_ap)
                    fillers.append((128, vt))
                return fillers

            def make_proj_fillers(tch):
                return (proj_part(tch, "q") + proj_part(tch, "k")
                        + proj_part(tch, "v") + vt_part(tch))

            # ---- attention emission machinery ----
            filler_q = []
            pending = []   # at most one (pv_fn, post_fn)

            def flush_pending():
                while pending:
                    pv, post = pending.pop(0)
                    pv()
                    if post:
                        post()

            def emit_unit(s_fn, pv_fn=None, post_fn=None, budget=0):
                s_fn()
                while budget > 0 and filler_q:
                    cost, f = filler_q.pop(0)
                    f()
                    budget -= cost
                if pending:
                    pv, post = pending.pop(0)
                    pv()
                    if post:
                        post()
                if pv_fn:
                    pending.append((pv_fn, post_fn))

            def flush_fillers():
                while filler_q:
                    filler_q.pop(0)[1]()

            def evict_dma(eng, b, jq, h, yn_ap, s):
                """Store one 256-col half of yn into A2A block 2jq+s."""
                csl = slice(s * 2 * TPC, (s + 1) * 2 * TPC)
                eng.dma_start(
                    ain[b][2 * jq + s, h * D:(h + 1) * D, :],
                    yn_ap[:, csl],
                )

            def mk_evict(b, jq, hstate, then=None):
                """Evict both heads' normalized y for (b, jq) into the
                A2A input blocks.  `then` (collective fire) marks a
                gating eviction: pipelined column halves on the scalar
                HWDGE queue."""
                def f():
                    for h in range(HL):
                        yt, den = hstate["yt"][h], hstate["den"][h]
                        bc = sb.tile([D, QCH], F32, tag="bc", bufs=3,
                                     name="bc")
                        rcp = sb.tile([D, QCH], F32, tag="rcp", bufs=3,
                                      name="rcp")
                        yn = sb.tile([D, QCH], BF, tag="yn", bufs=4,
                                     name="yn")
                        if then is None:
                            nc.gpsimd.partition_broadcast(bc[:], den[:])
                            nc.vector.reciprocal_approx_fast(rcp[:], bc[:])
                            nc.vector.tensor_mul(yn[:], yt[0:D, :], rcp[:])
                            for s in range(2):
                                evict_dma(nc.gpsimd, b, jq, h, yn, s)
                        else:
                            # gating: pipeline column halves, scalar DGE
                            for s in range(2):
                                csl = slice(s * 2 * TPC, (s + 1) * 2 * TPC)
                                nc.gpsimd.partition_broadcast(bc[:, csl],
                                                              den[:, csl])
                                nc.vector.reciprocal_approx_fast(
                                    rcp[:, csl], bc[:, csl])
                                nc.vector.tensor_mul(yn[:, csl],
                                                     yt[0:D, csl],
                                                     rcp[:, csl])
                                evict_dma(nc.scalar, b, jq, h, yn, s)
                    if then is not None:
                        then()
                return f

            def emit_pair(b, jq, then=None):
                """All nkt k-tile units for (b, jq), both heads packed."""
                q0 = b * T + jq * QCH
                nkt = 4 * jq + 4
                hstate = {}
                cells = [dict() for _ in range(nkt)]

                def mk_s(kt):
                    def f():
                        if kt == 0:
                            hstate["yt"] = [
                                ps.tile([D + 1, QCH], F32, tag="yt",
                                        bufs=2, name=f"yt{h}")
                                for h in range(HL)
                            ]
                            hstate["den"] = [
                                sb.tile([1, QCH], F32, tag="den", bufs=4,
                                        name=f"den{h}")
                                for h in range(HL)
                            ]
                        st = ps.tile([P, 2, QCH], F32, tag="st", bufs=2,
                                     name="st")
                        pt = sb.tile([P, 2, QCH], BF, tag="pt", bufs=4,
                                     name="pt")
                        cells[kt]["pt"] = pt
                        k0 = b * T + kt * P
                        i = kt - 4 * jq
                        qv = max(i, 0) * P
                        # both heads' S back-to-back: row groups (0,*)
                        # and (64,*) run concurrently on the PE array
                        for h in range(HL):
                            rsl = slice(h * D, (h + 1) * D)
                            nc.tensor.matmul(
                                st[:, h, qv:QCH],
                                kT_sb[rsl, k0:k0 + P],
                                qT_sb[rsl, q0 + qv:q0 + QCH],
                                start=True, stop=(i < 0),
                            )
                        if i >= 0:
                            for h in range(HL):
                                nc.tensor.matmul(
                                    st[:, h, qv:qv + P], ident[:], mneg[:],
                                    start=False, stop=True,
                                )
                        nc.scalar.activation(
                            pt[:, :, qv:], st[:, :, qv:], AF.Exp,
                            scale=float(SCALE)
                        )
                    return f

                def mk_pv(kt):
                    def f():
                        pt = cells[kt]["pt"]
                        qv = max(kt - 4 * jq, 0) * P
                        for h in range(HL):
                            nc.tensor.matmul(
                                hstate["yt"][h][:, qv:QCH],
                                v_sb[:, b * NKT + kt,
                                     h * (D + 1):(h + 1) * (D + 1)],
                                pt[:, h, qv:QCH],
                                start=(kt == 0), stop=(kt == nkt - 1),
                            )
                        if kt == nkt - 1:
                            for h in range(HL):
                                nc.vector.tensor_copy(
                                    hstate["den"][h][:],
                                    hstate["yt"][h][D:D + 1, :],
                                )
                    return f

                for kt in range(nkt):
                    qv = max(kt - 4 * jq, 0) * P
                    qvp = max(kt - 1 - 4 * jq, 0) * P
                    real = (QCH - qv) + (256 if kt >= 4 * jq else 0)
                    if kt > 0:
                        real += 2 * (QCH - qvp)   # pending PV pair
                    act = int(ACT_RATIO * (2 * (QCH - qv) + ACT_FIXED))
                    emit_unit(
                        mk_s(kt), mk_pv(kt),
                        mk_evict(b, jq, hstate, then) if kt == nkt - 1
                        else None,
                        budget=act - real,
                    )

            def a2a_fire(ain_t, aout_t):
                def f():
                    nc.gpsimd.collective_compute(
                        "AllToAll",
                        mybir.AluOpType.bypass,
                        replica_groups=[list(range(N_CORES))],
                        ins=[ain_t[:]],
                        outs=[aout_t[:]],
                    )
                return f

            yf_tiles = {}

            def yf_alloc(b):
                yf_tiles[b] = sb.tile([P, NCH, 2 * TPC], BF, tag="yf",
                                      bufs=2, name=f"yf{b}")

            def yf_clone(b):
                c = sb.tile([P, NCH, 2 * TPC], BF, tag="yfc", bufs=2,
                            name=f"yfc{b}")
                nc.vector.tensor_copy(c[:], yf_tiles[b][:])
                yf_tiles[b] = c

            def mk_po(b, om, t0, t1, eng):
                def f():
                    pot = ps.tile([P, QCH], F32, tag="fx", bufs=2,
                                  name="po")
                    po = pot[:, 0:t1 - t0]
                    yf = yf_tiles[b]
                    for ci in range(NCH):
                        nc.tensor.matmul(
                            po, wo_sb[:, om, ci, :], yf[:, ci, t0:t1],
                            start=(ci == 0), stop=(ci == NCH - 1),
                        )
                    obt = sb.tile([P, QCH], BF, tag="ob", bufs=2,
                                  name="ob")
                    ob = obt[:, 0:t1 - t0]
                    nc.vector.tensor_copy(ob, po)
                    eng.dma_start(out[:, om, b, t0:t1], ob)
                return f

            # ---- prologue: warms cover the w/x DMAs, then tch0 ----
            dain = dram.tile([N_CORES, 32], BF, name="dain")
            daout = dram.tile([N_CORES, 32], BF, name="daout")
            nc.sync.dma_start(dain[:], wsrc[0:8, 0:32])
            warm(6)
            filler_q.extend(make_proj_fillers(0))
            flush_fillers()
            # deferred Wo load, issued BEFORE the dummy collective
            # trigger: a trigger blocks the gpsimd queue until the CC
            # stream accepts it, but already-issued DMA transfers
            # proceed regardless (a trigger-first ordering froze the
            # vector queue behind wo_cp for ~80 us when the CC
            # bootstrap ran slow - measured 292 us outlier).
            for pc in range(4):
                csl = slice(WQKV + pc * (WO // 4),
                            WQKV + (pc + 1) * (WO // 4))
                nc.gpsimd.dma_start(w_sb[:, csl], wall[:, csl])
            # dummy bootstrap collective: absorbs the one-time CC
            # bootstrap + first-op rendezvous (~15-25 us) off the
            # critical path while attention runs
            a2a_fire(dain, daout)()

            # filler delivery map, balanced to each phase's slack
            # (phase (b,jq) has 4jq+4 units ~ 1k cycles of slack each)
            # and to the projection deadlines: q(t) before the phase
            # whose q-chunk is t; k/v(t) before that phase's diagonal
            # units (FIFO pop order guarantees both).
            fill_map = {
                (0, 0): make_proj_fillers(1),
                (0, 1): make_proj_fillers(2),
                (0, 2): make_proj_fillers(3),
                (0, 3): make_proj_fillers(4),
                (1, 0): make_proj_fillers(5),
                (1, 1): (proj_part(6, "q") + proj_part(6, "k")
                         + proj_part(6, "v") + vt_part(6)),
                (1, 2): (proj_part(7, "q") + proj_part(7, "v")
                         + proj_part(7, "k") + vt_part(7)),
                (1, 3): [],
            }
            # starved phases get keep-warm fillers: pure PE filler is
            # strictly better than the ~0.55us/unit ACT<->PE ping-pong
            # of a filler-less unit, and any real work here (po(b0))
            # loses a race against A2A(b0) completion (~140-152 us vs
            # (1,3) starting ~130)
            def warm_filler():
                wdst = ps.tile([P, QCH], F32, tag="fx", bufs=2,
                               name="wdst")
                nc.tensor.matmul(wdst[:], ident[:], wsrc[:],
                                 start=True, stop=True)
            fill_map[(0, 3)] += [(512, warm_filler)] * 6
            fill_map[(1, 3)] += [(512, warm_filler)] * 16

            for b in range(B):
                for jq in range(NQC):
                    filler_q.extend(fill_map[(b, jq)])
                    if (b, jq) == (1, 3):
                        # clone Wo here: the vector queue is idle in
                        # (1,3) (no projection fillers), while at
                        # (0,2)-(1,1) this 4us copy sat right in the
                        # boundary backlog that gates qT/v_sb copies
                        nc.vector.tensor_copy(wo_cp[:],
                                              w_sb[:, WQKV:WCOLS])
                    if (b, jq) == (1, 2):
                        # A2A(b0) done ~150: pull y(b0) into SBUF
                        yf_alloc(0)
                        nc.gpsimd.dma_start(
                            yf_tiles[0][:],
                            aout[0][:].rearrange("a p t -> p a t"),
                        )

                    then = None
                    if jq == NQC - 1:
                        then = a2a_fire(ain[b], aout[b])
                    emit_pair(b, jq, then)
                    flush_fillers()
            flush_pending()
            flush_fillers()

            # ---- tail: po(b0) + keep-warms bridge A2A(b1) (~15 us;
            # PE must not idle >3.4 us or HAM re-throttles and po(b1)
            # runs at half clock); then yf(b1) + po(b1) ----
            # po(b0) MUST be emitted before the yf(b1) loads: the
            # DMA-completion semaphore lanes are shared round-robin, so
            # anything emitted after yf(b1) picks up release thresholds
            # that transitively wait on A2A(b1) (measured -32 us).
            with tc.tile_wait_until(0.5):
                # re-materialize y(b0) via vector HERE (vector is idle
                # at the tail): doing it mid-phase stalls the whole
                # vector queue behind the A2A(b0)-gated DMA when that
                # collective runs slow (measured 257-277 us outliers)
                yf_clone(0)
                for om in range(NCH):
                    mk_po(0, om, 0, 2 * TPC, nc.scalar)()
                warm(16)
            with tc.tile_wait_until(0.51):
                yf_alloc(1)
                nc.sync.dma_start(
                    yf_tiles[1][:, 0:4, :],
                    aout[1][0:4].rearrange("a p t -> p a t"),
                )
                nc.sync.dma_start(
                    yf_tiles[1][:, 4:8, :],
                    aout[1][4:8].rearrange("a p t -> p a t"),
                )
            with tc.tile_wait_until(0.52):
                for om in range(NCH):
                    mk_po(1, om, 0, 2 * TPC, nc.sync)()

    nc.finalize()
    return nc


_GRAPH = None


def _get_graph():
    global _GRAPH
    if _GRAPH is None:
        _GRAPH = build_graph()
    return _GRAPH


def prepare_in_maps(x, Wq, Wk, Wv, Wo):
    x = np.asarray(x, np.float32)
    Wq = np.asarray(Wq, np.float32)
    Wk = np.asarray(Wk, np.float32)
    Wv = np.asarray(Wv, np.float32)
    Wo = np.asarray(Wo, np.float32)

    bf = ml_dtypes.bfloat16
    # [p, ci, t] with c = ci*128 + p
    xTh = np.ascontiguousarray(
        x.reshape(TT, NCH, P).transpose(2, 1, 0).reshape(P, NCH * TT)
    ).astype(bf)
    # full Wo packed [p, om, ci, m]: wo[p, om, ci, m] = Wo[om*128+m,
    # ci*128+p] (shared by all cores)
    woall = Wo.T.reshape(NCH, P, NCH, P).transpose(1, 2, 0, 3)
    woall = np.ascontiguousarray(woall.reshape(P, WO)).astype(bf)
    in_maps = []
    for r in range(N_CORES):
        sl = slice(r * DL, (r + 1) * DL)
        wqkv = np.empty((P, 3, NCH, DL), np.float32)
        for w, W in enumerate((Wq, Wk, Wv)):
            wqkv[:, w] = W[sl].T.reshape(NCH, P, DL).transpose(1, 0, 2)
        wallh = np.concatenate(
            [np.ascontiguousarray(wqkv.reshape(P, WQKV)).astype(bf), woall],
            axis=1,
        )
        in_maps.append({
            "xT": xTh,
            "wall": np.ascontiguousarray(wallh),
        })
    return in_maps


def assemble_output(results):
    outT = np.empty((B, C, T), np.float32)
    TSH = 2 * TPC  # 256 tokens per core per batch
    for r in range(N_CORES):
        o = np.asarray(results[r]["out"], np.float32)  # [m, om, b, 256]
        oT = o.transpose(2, 1, 0, 3).reshape(B, C, TSH)
        outT[:, :, r * TSH:(r + 1) * TSH] = oT
    return np.ascontiguousarray(outT.transpose(0, 2, 1))


def kernel(x, Wq, Wk, Wv, Wo):
    nc = _get_graph()
    in_maps = prepare_in_maps(x, Wq, Wk, Wv, Wo)
    res = run_bass_kernel_spmd(nc, in_maps, core_ids=list(range(N_CORES)))
    return assemble_output(res.results)
